# revision 8
# baseline (speedup 1.0000x reference)
"""CoverageLoss kernel for 8 Trainium2 NeuronCores.

Algorithm:
  loss = size(ls) + size(la) + cov(ss, ls) + cov(sa, la)
  cov(S, L): d = cdist_l1(S, L); sm4 = 4 smallest per row; tail = sm4.mean(-1)
             far = top64(tail); loss = mean(sm4[far]**2)

Device strategy (sample-sharded x8, latents replicated):
  Each core handles 256 samples per loss (2 tiles of 128) against all
  8192 latents.
  Phase A (TensorE): exact squared-L2 proxy scores via one K=e+1 f16
    matmul: score = 2*s.l - |l|^2 (row-constant |s|^2 omitted - ranking
    only).  Per 2048-latent scan chunk: 4 matmuls into one 4-bank PSUM
    tile -> one wide ACT copy-cast -> SBUF f16 chunk tile.
  Phase B (VectorE): per chunk, max8 + find_index8 -> top-8 proxy
    candidates; 4 chunks -> 32 candidates per sample (candidate depth
    drives recall of the true L1 4-NN).  f16 scans run ~1.6x faster
    than f32 on DVE.
  Phase C (exact): indirect-DMA gather of the 32 candidate latent rows
    (per-chunk tables, chunk-relative indices); exact fp32 L1 via
    tensor_tensor(sub) + tensor_reduce(abs-add).  Refine is emitted one
    tile behind the scans so gather latency hides under scan work.
  Size losses: per-row relu(|x|_1 - 1)^2 on a distinct 1024-row slice
    per core; emitted first to fill the pipeline-warmup hole.
Host: merge per-core [256, 32] exact candidate distances, sort, sm4,
  tails, global top-64, final scalar.
"""

from contextlib import ExitStack

import numpy as np

import concourse.bass as bass
import concourse.bacc as bacc
import concourse.mybir as mybir
import concourse.tile as tile
from concourse.bass_utils import run_bass_kernel_spmd

NLAT, ES, EA = 8192, 64, 32
NSMP = 2048
NCORES = 8
MS = NSMP // NCORES                # 256 samples per core per loss
NTILES = MS // 128                 # 2 sample tiles
SCAN_W = 2048                      # scan chunk width (= 4 PSUM banks)
NSCAN = NLAT // SCAN_W             # 4 scan chunks
MM_PER = SCAN_W // 512             # matmuls per scan chunk
NC_PER = 8                         # candidates per scan chunk
NCAND = NSCAN * NC_PER             # 32 candidates per sample
SZROWS = NLAT // NCORES            # 1024 size-loss rows per core

F32 = mybir.dt.float32
F16 = mybir.dt.float16
U32 = mybir.dt.uint32


def _cov_scan(tc, e, latT, sampT, lat_chunks, m, tag, pools, idx_out=None):
    """Matmul + evac + top8 scans + candidate gathers for one sample tile.

    Returns the gathered candidate rows tile [128, NCAND*e]."""
    nc = tc.nc
    psum, xpool, gpool, spool = pools

    gath = gpool.tile([128, NCAND * e], F32, tag=f"gath_{tag}")
    for c in range(NSCAN):
        ps = psum.tile([128, SCAN_W], F32, tag="ps")
        for k in range(MM_PER):
            nc.tensor.matmul(
                ps[:, k * 512:(k + 1) * 512],
                lhsT=sampT[:, m * 128:(m + 1) * 128],
                rhs=latT[:, c * SCAN_W + k * 512:c * SCAN_W + (k + 1) * 512],
                start=True, stop=True)
        xh = xpool.tile([128, SCAN_W], F16, tag="xh")
        nc.scalar.copy(xh[:], ps[:])

        m8 = spool.tile([128, 8], F16, tag="m8")
        nc.vector.max(out=m8[:], in_=xh[:])
        idx = spool.tile([128, 8], U32, tag="idx")
        nc.vector.max_index(out=idx[:], in_max=m8[:], in_values=xh[:])
        for k in range(NC_PER):
            nc.gpsimd.indirect_dma_start(
                out=gath[:, (c * NC_PER + k) * e:(c * NC_PER + k + 1) * e],
                out_offset=None,
                in_=lat_chunks[c][:, :],
                in_offset=bass.IndirectOffsetOnAxis(
                    ap=idx[:, k:k + 1], axis=0))
        if idx_out is not None:
            nc.sync.dma_start(
                idx_out[m * 128:(m + 1) * 128, c * 8:(c + 1) * 8], idx[:])
    return gath


def _cov_refine(tc, e, gath, smp_big, refc_out, m, tag, pools):
    """Exact L1 on gathered candidates -> refc_out rows for tile m."""
    nc = tc.nc
    psum, xpool, gpool, spool = pools
    smp_tile = smp_big[:, m * e:(m + 1) * e]
    diff = gpool.tile([128, NCAND * e], F32, tag=f"diff_{tag}")
    g3 = gath[:].rearrange("p (c e) -> p c e", c=NCAND)
    s3 = smp_tile[:, None, :].broadcast_to([128, NCAND, e])
    d3 = diff[:].rearrange("p (c e) -> p c e", c=NCAND)
    nc.vector.tensor_tensor(out=d3, in0=g3, in1=s3,
                            op=mybir.AluOpType.subtract)
    refc = spool.tile([128, NCAND], F32, tag=f"refc_{tag}")
    nc.vector.tensor_reduce(
        out=refc[:], in_=d3, axis=mybir.AxisListType.X,
        op=mybir.AluOpType.add, apply_absolute_value=True)
    nc.sync.dma_start(refc_out[m * 128:(m + 1) * 128, :], refc[:])


def _size_kernel(ctx, tc, e, lat_rows, sz_out, tag):
    """Per-row relu(|x|_1 - 1)^2 for a [SZROWS, e] slice -> [128, SZROWS//128]."""
    nc = tc.nc
    pool = ctx.enter_context(tc.tile_pool(name=f"sz_{tag}", bufs=1))
    nt = SZROWS // 128
    lat_big = pool.tile([128, nt * e], F32, tag=f"latbig_{tag}")
    nc.sync.dma_start(
        lat_big[:], lat_rows.rearrange("(m p) e -> p m e", p=128))
    norms = pool.tile([128, nt], F32, tag=f"norms_{tag}")
    nc.vector.tensor_reduce(
        out=norms[:], in_=lat_big[:].rearrange("p (m e) -> p m e", m=nt),
        axis=mybir.AxisListType.X, op=mybir.AluOpType.add,
        apply_absolute_value=True)
    rl = pool.tile([128, nt], F32, tag=f"rl_{tag}")
    nc.vector.tensor_scalar(out=rl[:], in0=norms[:], scalar1=1.0, scalar2=0.0,
                            op0=mybir.AluOpType.subtract,
                            op1=mybir.AluOpType.max)
    sq = pool.tile([128, nt], F32, tag=f"sq_{tag}")
    nc.vector.tensor_tensor(out=sq[:], in0=rl[:], in1=rl[:],
                            op=mybir.AluOpType.mult)
    nc.sync.dma_start(sz_out[:, :], sq[:])


def _build_nc():
    nc = bacc.Bacc("TRN2", target_bir_lowering=False, debug=False,
                   num_devices=8)
    inp = {}
    specs = [
        ("latT_s", [ES + 1, NLAT], F16), ("latT_a", [EA + 1, NLAT], F16),
        ("sampT_s", [ES + 1, MS], F16), ("sampT_a", [EA + 1, MS], F16),
        ("smp_s", [MS, ES], F32), ("smp_a", [MS, EA], F32),
        ("szin_s", [SZROWS, ES], F32), ("szin_a", [SZROWS, EA], F32),
    ]
    for c in range(NSCAN):
        specs.append((f"lat_s{c}", [SCAN_W, ES], F32))
        specs.append((f"lat_a{c}", [SCAN_W, EA], F32))
    for name, shape, dt in specs:
        inp[name] = nc.dram_tensor(name, shape, dt, kind="ExternalInput").ap()
    out = {}
    for name, shape in [
        ("refc_s", [MS, NCAND]), ("refc_a", [MS, NCAND]),
        ("idx_s", [MS, NCAND]), ("idx_a", [MS, NCAND]),
        ("szrows_s", [128, SZROWS // 128]), ("szrows_a", [128, SZROWS // 128]),
    ]:
        dt = U32 if name.startswith("idx") else F32
        out[name] = nc.dram_tensor(name, shape, dt, kind="ExternalOutput").ap()

    with tile.TileContext(nc) as tc:
        with ExitStack() as ctx:
            big = ctx.enter_context(tc.tile_pool(name="bigin", bufs=1))
            tiles = {}
            for name in ("latT_s", "latT_a", "sampT_s", "sampT_a"):
                t = big.tile(list(inp[name].shape), F16, tag=name)
                if name.startswith("latT"):
                    # column-chunked so chunk-c matmuls start early
                    for c in range(NSCAN):
                        nc.sync.dma_start(
                            t[:, c * SCAN_W:(c + 1) * SCAN_W],
                            inp[name][:, c * SCAN_W:(c + 1) * SCAN_W])
                else:
                    nc.sync.dma_start(t[:], inp[name][:, :])
                tiles[name] = t
            smp_bigs = {}
            for key, e in (("smp_s", ES), ("smp_a", EA)):
                t = big.tile([128, NTILES * e], F32, tag=f"big_{key}")
                nc.sync.dma_start(
                    t[:], inp[key].rearrange("(m p) e -> p m e", p=128))
                smp_bigs[key] = t

            # size losses first: they fill the pipeline warmup hole
            _size_kernel(ctx, tc, ES, inp["szin_s"], out["szrows_s"], "s")
            _size_kernel(ctx, tc, EA, inp["szin_a"], out["szrows_a"], "a")

            psum = ctx.enter_context(tc.tile_pool(name="psum", bufs=2,
                                                  space="PSUM"))
            xpool = ctx.enter_context(tc.tile_pool(name="xpool", bufs=4))
            gpool = ctx.enter_context(tc.tile_pool(name="gpool", bufs=2))
            spool = ctx.enter_context(tc.tile_pool(name="spool", bufs=8))
            pools = (psum, xpool, gpool, spool)

            lat_chunks = {
                "s": [inp[f"lat_s{c}"] for c in range(NSCAN)],
                "a": [inp[f"lat_a{c}"] for c in range(NSCAN)],
            }
            work = [("s", 0), ("s", 1), ("a", 0), ("a", 1)]
            args = {
                "s": (ES, tiles["latT_s"][:], tiles["sampT_s"][:],
                      smp_bigs["smp_s"][:], out["refc_s"]),
                "a": (EA, tiles["latT_a"][:], tiles["sampT_a"][:],
                      smp_bigs["smp_a"][:], out["refc_a"]),
            }
            pending = None
            for key, m in work:
                e, latT, sampT, smp_big, refc_out = args[key]
                gath = _cov_scan(tc, e, latT, sampT, lat_chunks[key], m,
                                 key, pools, idx_out=out[f"idx_{key}"])
                if pending is not None:
                    pkey, pm, pgath = pending
                    pe, _, _, psmp, prefc = args[pkey]
                    _cov_refine(tc, pe, pgath, psmp, prefc, pm,
                                pkey, pools)
                pending = (key, m, gath)
            pkey, pm, pgath = pending
            pe, _, _, psmp, prefc = args[pkey]
            _cov_refine(tc, pe, pgath, psmp, prefc, pm, pkey, pools)
    nc.compile()
    return nc


_NC_CACHE = {}


def _get_nc():
    if "nc" not in _NC_CACHE:
        _NC_CACHE["nc"] = _build_nc()
    return _NC_CACHE["nc"]


def _make_in_maps(latent_states, latent_actions, state_space_samples,
                  action_space_samples):
    def latT(L):
        # [e+1, NLAT] f16: rows 0..e-1 = 2*coord, row e = -|l|^2
        n2 = (L.astype(np.float64) ** 2).sum(-1).astype(np.float32)
        return np.ascontiguousarray(np.concatenate(
            [(2.0 * L).T.astype(np.float16),
             (-n2)[None, :].astype(np.float16)], 0))

    latT_s = latT(latent_states)
    latT_a = latT(latent_actions)
    shared = {"latT_s": latT_s, "latT_a": latT_a}
    for c in range(NSCAN):
        shared[f"lat_s{c}"] = np.ascontiguousarray(
            latent_states[c * SCAN_W:(c + 1) * SCAN_W])
        shared[f"lat_a{c}"] = np.ascontiguousarray(
            latent_actions[c * SCAN_W:(c + 1) * SCAN_W])

    in_maps = []
    for core in range(NCORES):
        smp_s = np.ascontiguousarray(state_space_samples[core*MS:(core+1)*MS])
        smp_a = np.ascontiguousarray(action_space_samples[core*MS:(core+1)*MS])
        sampT_s = np.ascontiguousarray(np.concatenate(
            [smp_s.T.astype(np.float16),
             np.ones((1, MS), np.float16)], 0))
        sampT_a = np.ascontiguousarray(np.concatenate(
            [smp_a.T.astype(np.float16),
             np.ones((1, MS), np.float16)], 0))
        m = dict(shared)
        m.update({
            "sampT_s": sampT_s, "sampT_a": sampT_a,
            "smp_s": smp_s, "smp_a": smp_a,
            "szin_s": np.ascontiguousarray(
                latent_states[core*SZROWS:(core+1)*SZROWS]),
            "szin_a": np.ascontiguousarray(
                latent_actions[core*SZROWS:(core+1)*SZROWS]),
        })
        in_maps.append(m)
    return in_maps


def _host_combine(results):
    """results: list of 8 per-core output dicts -> final scalar loss."""
    total = np.float64(0)
    sz_s = np.concatenate([r["szrows_s"].ravel() for r in results])
    sz_a = np.concatenate([r["szrows_a"].ravel() for r in results])
    total += sz_s.mean(dtype=np.float64)
    total += sz_a.mean(dtype=np.float64)
    for key in ("refc_s", "refc_a"):
        ref = np.concatenate([r[key] for r in results], 0)  # [NSMP, NCAND]
        ref.sort(axis=-1)
        sm4 = ref[:, :4]
        tails = sm4.mean(-1)
        far = np.argsort(-tails)[:64]
        total += np.float64((sm4[far].astype(np.float64) ** 2).mean())
    return np.float32(total)


def kernel(latent_states, latent_actions, state_space_samples,
           action_space_samples, _want_results=False, _trace=False):
    nc = _get_nc()
    in_maps = _make_in_maps(latent_states, latent_actions,
                            state_space_samples, action_space_samples)
    res = run_bass_kernel_spmd(nc, in_maps, core_ids=list(range(8)),
                               trace=_trace)
    out = _host_combine(res.results)
    if _want_results:
        return out, res
    return out
